# revision 5
# baseline (speedup 1.0000x reference)
"""Trainium2 Bass kernel: batched chamfer-style metric (nn_Metric_56985626083917).

Reference per batch b (B=8, N=M=4096, D=3):
    sqd[n,m] = |pred_n - gt_m|^2
    d1 = sqrt(min_m sqd)  [N] ; d2 = sqrt(min_n sqd)  [M]
    loss_b = mean(d1)+mean(d2) + 3*(mean(top2048(d1))+mean(top2048(d2)))
    out = mean_b loss_b

Strategy: data-parallel, one batch per NeuronCore (8 cores).

This runtime environment is dominated by a large per-instruction dispatch
cost (~40-70us per static instruction, ~220us per DRAM DMA), so the kernel
minimizes INSTRUCTION COUNT rather than element traffic:

Layout: gt on (partition, slot): gt index m = p*SL + s with SL=32 slots.
pred coords replicated along the free axis on every partition. Distances
are computed directly on the vector engine with broadcast access patterns
(stride-0 dims), f16 intermediates:

    per chunk k (C=1024 preds, 4 chunks):
      for c in xyz:  d = Pc - Gc (bcast TT), d = d*d, s += d
      reduce min over pred axis  -> run2 partial [128, 32]   (dist2 path)
      reduce min over slot axis  -> run1[:, chunk]  [128, C] (dist1 path)

Inputs arrive compact (one [128, 192] fp32 DRAM DMA: gt grid + pred packed
across partitions); pred is gathered to partition 0 and replicated to all
128 partitions by doubling SBUF->SBUF DMAs (cheap in this runtime, and the
small transfer keeps host-side wall time low and stable). Device ships
run1 [128, 4096] (per-partition mins; host folds the final 128-way min)
and run2 partials [128, 4x32] (host folds 4-way + reshape). Host: sqrt,
means, exact top-k, final scalar. ~50 instructions total, 2 DRAM DMAs +
8 SBUF DMAs.
"""

import os
import sys

import numpy as np

for _p in ("/opt/trn_rl_repo",):
    if os.path.isdir(_p) and _p not in sys.path:
        sys.path.insert(0, _p)

import concourse.bass as bass  # noqa: E402
import concourse.mybir as mybir  # noqa: E402
import concourse.tile as tile  # noqa: E402
from concourse import bacc  # noqa: E402
from concourse.bass_utils import run_bass_kernel_spmd  # noqa: E402

B = 8
NP = 4096  # pred points per batch
NG = 4096  # gt points per batch
P = 128  # partitions
SL = NG // P  # 32 gt slots per partition
CHUNKS = (1376, 1376, 1344)  # 3 unequal pred chunks (sum 4096)
CMAX = 1376
NCHUNK = len(CHUNKS)
K1 = NP // 2  # top-k count (PERCENT=0.5)
WEIGHT = 3.0

F16 = mybir.dt.float16
F32 = mybir.dt.float32
Alu = mybir.AluOpType

NCOLS_IN = 192  # 96 G-grid cols + 96 packed-pred cols (f16)
NCOLS_OUT = NP + NCHUNK * SL  # run1 then run2 partials

LAST_RESULT = None
_CACHE = {}


def _build_nc(reps=1):
    nc = bacc.Bacc(
        "TRN2", target_bir_lowering=False, debug=False, num_devices=B
    )
    inp = nc.dram_tensor("INP", [P, NCOLS_IN], F16, kind="ExternalInput")
    out = nc.dram_tensor("OUT", [P, NCOLS_OUT], F16, kind="ExternalOutput")

    with tile.TileContext(nc) as tc:
        for _ in range(reps):
            _body(nc, tc, inp, out)
    nc.compile()
    return nc


def _body(nc, tc, inp, out):
    from contextlib import ExitStack

    with ExitStack() as ctx:
        io = ctx.enter_context(tc.tile_pool(name="io", bufs=1))
        work = ctx.enter_context(tc.tile_pool(name="work", bufs=1))

        X = io.tile([P, NCOLS_IN], F16)
        nc.sync.dma_start(out=X, in_=inp[:])
        Gc = [X[:, c * SL : (c + 1) * SL] for c in range(3)]
        # gather packed pred (128 partitions x 96 cols) into partition 0,
        # then replicate to all partitions by doubling (SBUF->SBUF DMAs).
        Prep = io.tile([P, 3 * NP], F16, name="Prep")
        nc.sync.dma_start(out=Prep[0:1, :], in_=X[:, 96:192])
        npart = 1
        while npart < P:
            nc.sync.dma_start(
                out=Prep[npart : 2 * npart, :], in_=Prep[0:npart, :]
            )
            npart *= 2
        Pc = [Prep[:, c * NP : (c + 1) * NP] for c in range(3)]

        RUN = io.tile([P, NCOLS_OUT], F16, name="RUN")
        run1 = RUN[:, :NP]
        run2 = RUN[:, NP:]

        d16 = work.tile([P, SL, CMAX], F16, name="d16")
        s16 = work.tile([P, SL, CMAX], F16, name="s16")

        off = 0
        for k, C in enumerate(CHUNKS):
            sl = slice(off, off + C)
            dk = d16[:, :, :C]
            sk = s16[:, :, :C]
            for c in range(3):
                pv = Pc[c][:, sl].unsqueeze(1).broadcast_to([P, SL, C])
                gv = Gc[c][:, :].unsqueeze(2).broadcast_to([P, SL, C])
                if c == 0:
                    nc.vector.tensor_tensor(dk, pv, gv, op=Alu.subtract)
                    nc.vector.tensor_tensor(sk, dk, dk, op=Alu.mult)
                else:
                    nc.vector.tensor_tensor(dk, pv, gv, op=Alu.subtract)
                    nc.vector.tensor_tensor(dk, dk, dk, op=Alu.mult)
                    nc.vector.tensor_tensor(sk, sk, dk, op=Alu.add)
            # dist2 partial: min over pred axis (innermost) -> [P, SL]
            nc.vector.tensor_reduce(
                out=run2[:, k * SL : (k + 1) * SL],
                in_=sk,
                axis=mybir.AxisListType.X,
                op=Alu.min,
            )
            # dist1: min over slots -> [P, C] straight into run1 slice
            nc.vector.tensor_reduce(
                out=run1[:, sl],
                in_=sk[:].transpose([0, 2, 1]),
                axis=mybir.AxisListType.X,
                op=Alu.min,
            )
            off += C

        nc.sync.dma_start(out=out[:], in_=RUN)


def _prep(pred, gt):
    """Build the packed [128, NCOLS_IN] fp32 input for one batch."""
    pred = np.asarray(pred, np.float32)
    gt = np.asarray(gt, np.float32)
    X = np.empty((P, NCOLS_IN), np.float16)
    for c in range(3):
        X[:, c * SL : (c + 1) * SL] = gt[:, c].reshape(P, SL)
    predcols = np.concatenate([pred[:, 0], pred[:, 1], pred[:, 2]])
    X[:, 96:192] = predcols.reshape(P, 96)
    return {"INP": X}


def _in_maps(inputs):
    pred_pc = np.asarray(inputs["pred_pc"])
    gt_pc = np.asarray(inputs["gt_pc"])
    return [_prep(pred_pc[b], gt_pc[b]) for b in range(B)]


def _get_nc():
    if "nc" not in _CACHE:
        _CACHE["nc"] = _build_nc()
    return _CACHE["nc"]


def _finish(res):
    losses = []
    for b in range(B):
        OUT = np.asarray(res.results[b]["OUT"], np.float32)
        run1 = OUT[:, :NP]
        run2 = OUT[:, NP:].reshape(P, NCHUNK, SL)
        d1 = np.sqrt(np.maximum(run1.min(axis=0), 0.0))  # [NP]
        d2 = np.sqrt(np.maximum(run2.min(axis=1).reshape(-1), 0.0))  # [NG]
        loss = 0.0
        for d in (d1, d2):
            k = d.size // 2
            topk = np.partition(d, d.size - k)[d.size - k :]
            loss += d.mean() + WEIGHT * topk.mean()
        losses.append(loss)
    return np.array(np.mean(losses), dtype=np.float32)


def kernel(pred_pc, gt_pc):
    global LAST_RESULT
    nc = _get_nc()
    in_maps = _in_maps({"pred_pc": pred_pc, "gt_pc": gt_pc})
    res = run_bass_kernel_spmd(nc, in_maps, list(range(B)))
    LAST_RESULT = res
    return _finish(res)
